# revision 19
# baseline (speedup 1.0000x reference)
"""Trainium2 Bass kernel for nn_Attention_6313601925220 (sparse_attention).

Reference computation (per (b,h) head; K == Q):
    QR = rope(Q)                      # interleaved-pair RoPE, phases = t * freqs[n]
    scores = tril(QR @ QR^T, k=-1)    # strictly causal, NO softmax
    out = scores @ V

No softmax => the strictly-causal masked product is linear; computed with the
chunked linear-attention prefix scan:
    P_i = sum_{j<i} QR_j^T V_j                  # [N, DV] running state (PSUM, f32)
    out_i = QR_i @ P_i + tril_strict(QR_i QR_i^T) @ V_i

v2 design (cost-model driven):
  - RoPE in even/odd-split form: the host permutes Q's feature axis to
    [even | odd] halves; freqs are pair-quantized (floor(i/2)*2, per the
    reference's _get_freqs), so cos/sin tables collapse to half width:
        qrE = qE*c - qO*s ; qrO = qO*c + qE*s      (c,s = pair tables)
    6 ops of [128, G*128] per G-chunk group, all eligible for DVE 2x mode.
    The E/O relabeling is a global permutation of the contraction axis n, so
    scores and P are unchanged as long as it is applied consistently.
  - Engine budget (per 64 head-chunks): PE 832 rows/chunk (transposes 256,
    ST 256, intra 64, inter 128, P-update 128) ~= 22.2us.  Elementwise split:
    DVE = rope (most slots), Pool = rope leftovers + mask-evac + P-evac
    (pair-combined, [128,256] each), Act = qrt evac (batched x4 chunks,
    [128,1024]) + out evac ([128,512] per 8 chunks).
  - DMA: per-instruction trigger cost dominates (~500-790ns serial on SP;
    transfers serialize at ~360GB/s when the contiguous run >= 512B).  All
    tensors are host-retiled to direct SBUF images ([128, free]) so every
    transfer runs at full descriptor width, in ~30 need-ordered triggers.
  - PSUM: qrt 2 banks (bf16, 2x2-chunk batch), ST 2 banks (f32 pair tiles),
    out accumulators 3 banks, P (both heads combined) 1 bank.  start=True
    clears has_written for a whole 2KB bank; values persist and cleared
    regions are overwritten by the next write (HW-validated in the previous
    session), which makes the shared-bank P/ST packing safe with in-order PE.

Sharding: B*NH = 32 heads, 4 heads per core across 8 cores, fully
independent - no collectives.
"""

import os
import math

os.environ.setdefault("MYCRO_LOCAL_CACHE", "1")

import numpy as np
import ml_dtypes

from contextlib import ExitStack

import concourse.bass as bass
import concourse.tile as tile
from concourse import bacc, mybir
from concourse.bass_utils import run_bass_kernel_spmd

# Problem shapes (hardcoded per spec)
B, NH, T, N, DV = 2, 16, 2048, 256, 64
NCORES = 8
BH = B * NH              # 32 heads total
HPC = BH // NCORES       # 4 heads per core
CH = 128                 # chunk length along t
NCH = T // CH            # 16 chunks per head
NP = N // 2              # 128 rotation pairs

F32 = mybir.dt.float32
BF16 = mybir.dt.bfloat16
NPBF16 = ml_dtypes.bfloat16

# rope groups (start_chunk, n_chunks) per head; pass-0 heads start finer so
# compute begins as soon as the first small DMA pieces land.
GROUPS_P0 = [(0, 2), (2, 2), (4, 4), (8, 8)]
GROUPS_P1 = [(0, 8), (8, 8)]


def _build_nc():
    nc = bacc.Bacc(None, target_bir_lowering=False)

    q_d = nc.dram_tensor("q", [128, HPC * NCH * N], BF16, kind="ExternalInput")
    v_d = nc.dram_tensor("v", [128, HPC * NCH * DV], BF16, kind="ExternalInput")
    c_d = nc.dram_tensor("ctab", [128, NCH * NP], BF16, kind="ExternalInput")
    s_d = nc.dram_tensor("stab", [128, NCH * NP], BF16, kind="ExternalInput")
    o_d = nc.dram_tensor("out", [128, HPC * NCH * DV], BF16, kind="ExternalOutput")

    ident_d = nc.inline_tensor(np.eye(128).astype(NPBF16), "ident_c")
    # ST layout is [s, tq]; keep strictly-causal entries s < tq -> strict
    # upper; tiled x4 for the block evacuation (2 chunks x 2 heads).
    mask4_d = nc.inline_tensor(
        np.tile(np.triu(np.ones((128, 128)), k=1), (1, 4)).astype(NPBF16), "mask4_c")

    with tile.TileContext(nc) as tc, ExitStack() as ctx:
        consts = ctx.enter_context(tc.tile_pool(name="consts", bufs=1))
        ropep = ctx.enter_context(tc.tile_pool(name="rope", bufs=10))
        qrp = ctx.enter_context(tc.tile_pool(name="qr", bufs=6))
        qrtp = ctx.enter_context(tc.tile_pool(name="qrt", bufs=3))
        stp = ctx.enter_context(tc.tile_pool(name="stsb", bufs=4))
        pp = ctx.enter_context(tc.tile_pool(name="psb", bufs=4))
        ps_qrt = ctx.enter_context(tc.tile_pool(name="ps_qrt", bufs=3, space="PSUM"))
        ps_st = ctx.enter_context(tc.tile_pool(name="ps_st", bufs=2, space="PSUM"))
        ps_o = ctx.enter_context(tc.tile_pool(name="ps_o", bufs=1, space="PSUM"))
        ps_p = ctx.enter_context(tc.tile_pool(name="ps_p", bufs=1, space="PSUM"))

        ident = consts.tile([128, 128], BF16, tag="ident")
        mask4 = consts.tile([128, 512], BF16, tag="mask4")

        qsb = [consts.tile([128, NCH * N], BF16, tag=f"q{h}", name=f"q{h}")
               for h in range(HPC)]
        vsb = [consts.tile([128, NCH * DV], BF16, tag=f"v{h}", name=f"v{h}")
               for h in range(HPC)]
        ctab = consts.tile([128, NCH * NP], BF16, tag="ctab")
        stab = consts.tile([128, NCH * NP], BF16, tag="stab")
        osb = consts.tile([128, HPC * NCH * DV], BF16, tag="osb")

        def load_q(h, c0, cl):
            lo, hi = (h * NCH + c0) * N, (h * NCH + c0 + cl) * N
            nc.sync.dma_start(qsb[h][:, c0 * N:(c0 + cl) * N], q_d[:, lo:hi])

        def load_v(h, c0, cl):
            lo, hi = (h * NCH + c0) * DV, (h * NCH + c0 + cl) * DV
            nc.sync.dma_start(vsb[h][:, c0 * DV:(c0 + cl) * DV], v_d[:, lo:hi])

        def load_tab(c0, cl):
            nc.sync.dma_start(ctab[:, c0 * NP:(c0 + cl) * NP],
                              c_d[:, c0 * NP:(c0 + cl) * NP])
            nc.sync.dma_start(stab[:, c0 * NP:(c0 + cl) * NP],
                              s_d[:, c0 * NP:(c0 + cl) * NP])

        # need-ordered loads (SP serial): early pieces small, later big.
        load_tab(0, 4)
        load_q(0, 0, 2)
        load_q(1, 0, 2)
        nc.sync.dma_start(ident[:, :], ident_d[:, :])
        load_v(0, 0, 4)
        load_v(1, 0, 4)
        load_q(0, 2, 2)
        load_q(1, 2, 2)
        nc.sync.dma_start(mask4[:, :], mask4_d[:, :])
        load_q(0, 4, 4)
        load_q(1, 4, 4)
        load_tab(4, 12)
        load_v(0, 4, 12)
        load_v(1, 4, 12)
        load_q(0, 8, 8)
        load_q(1, 8, 8)
        for h in (2, 3):
            load_q(h, 0, 8)
            load_v(h, 0, 16)
        for h in (2, 3):
            load_q(h, 8, 8)

        ctv = ctab[:, :].rearrange("p (c k) -> p c k", c=NCH)
        stv = stab[:, :].rearrange("p (c k) -> p c k", c=NCH)

        # rope engine schedule: 6 op slots per group
        #   [m1=qE*c, m2=qO*s, m3=qO*c, m4=qE*s, qrE=m1-m2, qrO=m3+m4]
        # DVE is cheapest (2x mode); Pool takes ~1.25 slots on average.
        rope_ctr = [0]

        def emit_rope(h, c0, cl, qr_tile):
            g = rope_ctr[0]
            rope_ctr[0] += 1
            qv = qsb[h][:, :].rearrange("p (c n) -> p c n", c=NCH)
            qE = qv[:, c0:c0 + cl, 0:NP]
            qO = qv[:, c0:c0 + cl, NP:N]
            cv = ctv[:, c0:c0 + cl, :]
            sv = stv[:, c0:c0 + cl, :]
            qrv = qr_tile[:, :].rearrange("p (c e k) -> p c e k", c=cl, e=2)
            qrE = qrv[:, :, 0, :]
            qrO = qrv[:, :, 1, :]

            def mt(tag):
                t = ropep.tile([128, cl * NP], BF16, tag=tag)
                return t[:, :].rearrange("p (c k) -> p c k", c=cl)

            m1, m2, m3, m4 = mt("m1"), mt("m2"), mt("m3"), mt("m4")
            # GPSIMD cannot touch PSUM, so Pool only ever does rope; give it
            # ~4 of the 6 slots (every 8th group one back to DVE).
            dve_m3 = (g % 8 == 7)
            nc.gpsimd.tensor_mul(m1, qE, cv)
            nc.vector.tensor_mul(m2, qO, sv)
            if dve_m3:
                nc.vector.tensor_mul(m3, qO, cv)
            else:
                nc.gpsimd.tensor_mul(m3, qO, cv)
            nc.gpsimd.tensor_mul(m4, qE, sv)
            nc.vector.tensor_sub(qrE, m1, m2)
            nc.gpsimd.tensor_add(qrO, m3, m4)

        # Global rope emission plan.  Slots: pass0 prologue-top=0,
        # prologue-bottom=1, iteration j bottom=2+j; pass1 shifted by 10.
        # Pass-1 groups are emitted EARLY (during pass-0 iterations, after
        # their q DMA lands) so Pool/DVE never sit on rope work at the end
        # and the pipeline drain stays short.
        rope_plan = {
            0: [(0, 0, 2), (0, 2, 2)],
            1: [(0, 4, 4)],
            3: [(0, 8, 8)],
            6: [(1, 0, 8)],
            8: [(1, 8, 8)],
        }
        qr_tiles = {}          # (pass_i, k, c) -> (tile, c0)
        qr_seq = [0]

        def emit_rope_slot(s):
            for (pi, c0, cl) in rope_plan.get(s, []):
                for k in range(2):
                    h = 2 * pi + k
                    t = qrp.tile([128, cl * N], BF16, tag=f"qr{k}",
                                 name=f"qr_{pi}_{k}_{c0}_{qr_seq[0]}")
                    qr_seq[0] += 1
                    emit_rope(h, c0, cl, t)
                    for c in range(c0, c0 + cl):
                        qr_tiles[(pi, k, c)] = (t, c0)

        # per pass: heads (2p, 2p+1) chunk-locked
        for pass_i in (0, 1):
            heads = (2 * pass_i, 2 * pass_i + 1)
            slot_base = pass_i * 10

            def qr_slice(k, c, half):
                t, c0 = qr_tiles[(pass_i, k, c)]
                v = t[:, :].rearrange("p (c e k) -> p c e k", c=(t.shape[1] // N), e=2)
                return v[:, c - c0, half, :]

            # transposes of block j (chunks 2j, 2j+1, both heads) go into one
            # 2KB bf16 psum bank; the evac is emitted separately so p-evacs
            # are never queued behind it on Act.
            qrt_sb = {}        # block -> sbuf tile [128, 1024]
            qrt_ps_t = {}      # block -> psum tile

            def emit_transposes(j):
                ps = ps_qrt.tile([128, 1024], BF16, tag="qrt_ps",
                                 name=f"qrtps_{pass_i}_{j}")
                for ci, c in enumerate((2 * j, 2 * j + 1)):
                    for k in range(2):
                        for half in range(2):
                            off = ((ci * 2 + k) * 2 + half) * 128
                            nc.tensor.matmul(
                                ps[:, off:off + 128], lhsT=qr_slice(k, c, half),
                                rhs=ident[:, :], is_transpose=True,
                                start=True, stop=True)
                qrt_ps_t[j] = ps

            def emit_qrt_evac(j):
                sb = qrtp.tile([128, 1024], BF16, tag="qrt_sb",
                               name=f"qrtsb_{pass_i}_{j}")
                if j % 2 == 0:
                    nc.scalar.copy(sb[:, :], qrt_ps_t[j][:, :])
                else:
                    nc.vector.tensor_copy(sb[:, :], qrt_ps_t[j][:, :])
                qrt_sb[j] = sb

            def qrt_slice(k, c, half):
                j = c // 2
                ci = c % 2
                off = ((ci * 2 + k) * 2 + half) * 128
                return qrt_sb[j][:, off:off + 128]

            p_ps = ps_p.tile([128, 256], F32, tag="pps", name=f"pps{pass_i}")
            o8 = [None, None]
            p_sb = {}          # chunk -> sbuf tile holding P after that chunk
            st_sb = {}         # block -> masked bf16 scores [128, 512]
            st_ps_t = {}       # block -> raw f32 scores in psum

            def emit_ST(j):
                st_ps = ps_st.tile([128, 512], F32, tag="st_ps",
                                   name=f"stps_{pass_i}_{j}")
                for ci, c in enumerate((2 * j, 2 * j + 1)):
                    for k in range(2):
                        sl = st_ps[:, (ci * 2 + k) * 128:(ci * 2 + k + 1) * 128]
                        nc.tensor.matmul(sl, lhsT=qrt_slice(k, c, 0),
                                         rhs=qrt_slice(k, c, 0),
                                         start=True, stop=False)
                        nc.tensor.matmul(sl, lhsT=qrt_slice(k, c, 1),
                                         rhs=qrt_slice(k, c, 1),
                                         start=False, stop=True)
                st_ps_t[j] = st_ps

            def emit_mask(j):
                sb = stp.tile([128, 512], BF16, tag="st_sb",
                              name=f"stsb_{pass_i}_{j}")
                nc.vector.tensor_mul(sb[:, :], st_ps_t[j][:, :], mask4[:, :])
                st_sb[j] = sb

            # P += QR_c^T V_c, both heads (shared bank, long-open group; only
            # the very first matmul of the pass starts it), then the pair P
            # evacuation on Act.  The p-evac -> next P-update WAR round trip
            # is the critical ring; callers place a full block of independent
            # PE work between consecutive emit_P calls.
            def emit_P(c):
                first = c == 0
                last = c == NCH - 1
                for k, h in enumerate(heads):
                    vi = vsb[h][:, c * DV:(c + 1) * DV]
                    for half in range(2):
                        reg = p_ps[:, k * 128 + half * 64:k * 128 + (half + 1) * 64]
                        nc.tensor.matmul(
                            reg, lhsT=qr_slice(k, c, half), rhs=vi,
                            start=(first and k == 0 and half == 0),
                            stop=last, skip_group_check=True)
                if not last:
                    p_new = pp.tile([128, 256], BF16, tag="p_sb",
                                    name=f"psb_{pass_i}_{c}")
                    nc.scalar.copy(p_new[:, :], p_ps[:, :])
                    p_sb[c] = p_new

            # out accumulation for block j (intra + inter) and the per-head
            # out evacuation + store.
            def emit_stage2(j):
                for ci, c in enumerate((2 * j, 2 * j + 1)):
                    first = c == 0
                    for k, h in enumerate(heads):
                        vi = vsb[h][:, c * DV:(c + 1) * DV]
                        if c % 8 == 0:
                            o8[k] = ps_o.tile([128, 512], F32, tag=f"o8_{k}",
                                              name=f"o8_{pass_i}_{k}_{c}")
                        o_sl = o8[k][:, (c % 8) * DV:(c % 8 + 1) * DV]
                        stm = st_sb[j][:, (ci * 2 + k) * 128:(ci * 2 + k + 1) * 128]
                        nc.tensor.matmul(o_sl, lhsT=stm, rhs=vi,
                                         start=True, stop=first)
                        if not first:
                            for half in range(2):
                                pv = p_sb[c - 1][:, k * 128 + half * 64:
                                                 k * 128 + (half + 1) * 64]
                                nc.tensor.matmul(
                                    o_sl, lhsT=qrt_slice(k, c, half), rhs=pv,
                                    start=False, stop=(half == 1),
                                    skip_group_check=True)
                        if c % 8 == 7:
                            g8 = c // 8
                            base = (h * NCH + g8 * 8) * DV
                            nc.scalar.copy(osb[:, base:base + 512], o8[k][:, :])
                        if c == NCH - 1:
                            base = h * NCH * DV
                            nc.sync.dma_start(o_d[:, base:base + 1024],
                                              osb[:, base:base + 1024])

            # software pipeline; PE stream per iteration j:
            #   T(j+2), ST(j+1), P(2j+2), intra/inter(j), P(2j+3)
            # so each P-update ring round-trip hides under independent work.
            emit_rope_slot(slot_base + 0)
            emit_transposes(0)
            emit_transposes(1)
            emit_qrt_evac(0)
            emit_qrt_evac(1)
            emit_ST(0)
            emit_mask(0)
            emit_P(0)
            emit_P(1)
            emit_rope_slot(slot_base + 1)
            NB = NCH // 2
            for j in range(NB):               # 8 blocks of 2 chunks
                if j + 2 < NB:
                    emit_transposes(j + 2)
                if j + 1 < NB:
                    emit_ST(j + 1)
                    emit_mask(j + 1)
                if j + 2 < NB:
                    emit_qrt_evac(j + 2)
                if j + 1 < NB:
                    emit_P(2 * j + 2)
                emit_stage2(j)
                if j + 1 < NB:
                    emit_P(2 * j + 3)
                emit_rope_slot(slot_base + 2 + j)

    nc.finalize()
    return nc


_NC = None


def _get_nc():
    global _NC
    if _NC is None:
        _NC = _build_nc()
    return _NC


def _host_prep(Q, V, freqs):
    """Host-side retiling to direct SBUF images.

    - Q feature axis permuted to [even | odd] halves (global relabeling of the
      contraction axis; scores/P invariant).
    - cos/sin pair tables [T, 128] (freqs are pair-quantized in the reference:
      floor(i/2)*2, so cos/sin agree within each (2i, 2i+1) pair).
    - every tensor stored as [128, free] so each DMA row is one contiguous
      descriptor run.
    """
    Qf = np.asarray(Q, dtype=np.float32).reshape(BH, T, N)
    Vf = np.asarray(V, dtype=np.float32).reshape(BH, T, DV)
    f = np.asarray(freqs, dtype=np.float32).reshape(N)

    t = np.arange(T, dtype=np.float32).reshape(T, 1)
    ang = np.mod(t * f.reshape(1, N), 1.0).astype(np.float32) * np.float32(2.0 * math.pi)
    ce = np.cos(ang[:, 0::2]).astype(NPBF16)     # [T, 128]
    se = np.sin(ang[:, 0::2]).astype(NPBF16)
    ctab = ce.reshape(NCH, CH, NP).transpose(1, 0, 2).reshape(128, NCH * NP)
    stab = se.reshape(NCH, CH, NP).transpose(1, 0, 2).reshape(128, NCH * NP)

    perm = np.concatenate([np.arange(0, N, 2), np.arange(1, N, 2)])
    Qp = Qf[:, :, perm].astype(NPBF16)           # [BH, T, N] -> E|O halves
    Vb = Vf.astype(NPBF16)

    q_cores = []
    v_cores = []
    for c in range(NCORES):
        hs = slice(c * HPC, (c + 1) * HPC)
        qc = Qp[hs].reshape(HPC, NCH, CH, N).transpose(2, 0, 1, 3).reshape(
            128, HPC * NCH * N)
        vc = Vb[hs].reshape(HPC, NCH, CH, DV).transpose(2, 0, 1, 3).reshape(
            128, HPC * NCH * DV)
        q_cores.append(np.ascontiguousarray(qc))
        v_cores.append(np.ascontiguousarray(vc))
    return q_cores, v_cores, np.ascontiguousarray(ctab), np.ascontiguousarray(stab)


def _run(inputs, trace=False, trace_kwargs=None):
    q_cores, v_cores, ctab, stab = _host_prep(
        inputs["Q"], inputs["V"], inputs["freqs"])

    in_maps = []
    for c in range(NCORES):
        in_maps.append({
            "q": q_cores[c],
            "v": v_cores[c],
            "ctab": ctab,
            "stab": stab,
        })

    nc = _get_nc()
    kw = {}
    if trace:
        kw = dict(trace=True, trace_kwargs=trace_kwargs or {})
    res = run_bass_kernel_spmd(nc, in_maps, core_ids=list(range(NCORES)), **kw)

    out = np.empty((BH, T, DV), dtype=np.float32)
    for c in range(NCORES):
        oc = res.results[c]["out"].astype(np.float32)        # [128, HPC*NCH*DV]
        oc = oc.reshape(128, HPC, NCH, DV).transpose(1, 2, 0, 3)
        out[c * HPC:(c + 1) * HPC] = oc.reshape(HPC, T, DV)
    return out.reshape(B, NH, T, DV), res


def kernel(**inputs):
    out, _ = _run(inputs, trace=False)
    return out


# revision 23
# speedup vs baseline: 1.0019x; 1.0019x over previous
"""Trainium2 Bass kernel for nn_Attention_6313601925220 (sparse_attention).

Reference computation (per (b,h) head; K == Q):
    QR = rope(Q)                      # interleaved-pair RoPE, phases = t * freqs[n]
    scores = tril(QR @ QR^T, k=-1)    # strictly causal, NO softmax
    out = scores @ V

No softmax => the strictly-causal masked product is linear; computed with the
chunked linear-attention prefix scan:
    P_i = sum_{j<i} QR_j^T V_j                  # [N, DV] running state (PSUM, f32)
    out_i = QR_i @ P_i + tril_strict(QR_i QR_i^T) @ V_i

v2 design (cost-model driven):
  - RoPE in even/odd-split form: the host permutes Q's feature axis to
    [even | odd] halves; freqs are pair-quantized (floor(i/2)*2, per the
    reference's _get_freqs), so cos/sin tables collapse to half width:
        qrE = qE*c - qO*s ; qrO = qO*c + qE*s      (c,s = pair tables)
    6 ops of [128, G*128] per G-chunk group, all eligible for DVE 2x mode.
    The E/O relabeling is a global permutation of the contraction axis n, so
    scores and P are unchanged as long as it is applied consistently.
  - Engine budget (per 64 head-chunks): PE 832 rows/chunk (transposes 256,
    ST 256, intra 64, inter 128, P-update 128) ~= 22.2us.  Elementwise split:
    DVE = rope (most slots), Pool = rope leftovers + mask-evac + P-evac
    (pair-combined, [128,256] each), Act = qrt evac (batched x4 chunks,
    [128,1024]) + out evac ([128,512] per 8 chunks).
  - DMA: per-instruction trigger cost dominates (~500-790ns serial on SP;
    transfers serialize at ~360GB/s when the contiguous run >= 512B).  All
    tensors are host-retiled to direct SBUF images ([128, free]) so every
    transfer runs at full descriptor width, in ~30 need-ordered triggers.
  - PSUM: qrt 2 banks (bf16, 2x2-chunk batch), ST 2 banks (f32 pair tiles),
    out accumulators 3 banks, P (both heads combined) 1 bank.  start=True
    clears has_written for a whole 2KB bank; values persist and cleared
    regions are overwritten by the next write (HW-validated in the previous
    session), which makes the shared-bank P/ST packing safe with in-order PE.

Sharding: B*NH = 32 heads, 4 heads per core across 8 cores, fully
independent - no collectives.
"""

import os
import math

os.environ.setdefault("MYCRO_LOCAL_CACHE", "1")

import numpy as np
import ml_dtypes

from contextlib import ExitStack

import concourse.bass as bass
import concourse.tile as tile
from concourse import bacc, mybir
from concourse.bass_utils import run_bass_kernel_spmd

# Problem shapes (hardcoded per spec)
B, NH, T, N, DV = 2, 16, 2048, 256, 64
NCORES = 8
BH = B * NH              # 32 heads total
HPC = BH // NCORES       # 4 heads per core
CH = 128                 # chunk length along t
NCH = T // CH            # 16 chunks per head
NP = N // 2              # 128 rotation pairs

F32 = mybir.dt.float32
BF16 = mybir.dt.bfloat16
NPBF16 = ml_dtypes.bfloat16

# rope groups (start_chunk, n_chunks) per head; pass-0 heads start finer so
# compute begins as soon as the first small DMA pieces land.
GROUPS_P0 = [(0, 2), (2, 2), (4, 4), (8, 8)]
GROUPS_P1 = [(0, 8), (8, 8)]


def _build_nc():
    nc = bacc.Bacc(None, target_bir_lowering=False)

    q_d = nc.dram_tensor("q", [128, HPC * NCH * N], BF16, kind="ExternalInput")
    v_d = nc.dram_tensor("v", [128, HPC * NCH * DV], BF16, kind="ExternalInput")
    c_d = nc.dram_tensor("ctab", [128, NCH * NP], BF16, kind="ExternalInput")
    s_d = nc.dram_tensor("stab", [128, NCH * NP], BF16, kind="ExternalInput")
    o_d = nc.dram_tensor("out", [128, HPC * NCH * DV], BF16, kind="ExternalOutput")

    ident_d = nc.inline_tensor(np.eye(128).astype(NPBF16), "ident_c")
    # ST layout is [s, tq]; keep strictly-causal entries s < tq -> strict
    # upper; tiled x4 for the block evacuation (2 chunks x 2 heads).
    mask4_d = nc.inline_tensor(
        np.tile(np.triu(np.ones((128, 128)), k=1), (1, 4)).astype(NPBF16), "mask4_c")

    with tile.TileContext(nc) as tc, ExitStack() as ctx:
        consts = ctx.enter_context(tc.tile_pool(name="consts", bufs=1))
        ropep = ctx.enter_context(tc.tile_pool(name="rope", bufs=6))
        qrp = ctx.enter_context(tc.tile_pool(name="qr", bufs=6))
        qrtp = ctx.enter_context(tc.tile_pool(name="qrt", bufs=4))
        stp = ctx.enter_context(tc.tile_pool(name="stsb", bufs=6))
        pp = ctx.enter_context(tc.tile_pool(name="psb", bufs=6))
        ps_qrt = ctx.enter_context(tc.tile_pool(name="ps_qrt", bufs=3, space="PSUM"))
        ps_st = ctx.enter_context(tc.tile_pool(name="ps_st", bufs=2, space="PSUM"))
        ps_o = ctx.enter_context(tc.tile_pool(name="ps_o", bufs=1, space="PSUM"))
        ps_p = ctx.enter_context(tc.tile_pool(name="ps_p", bufs=1, space="PSUM"))

        ident = consts.tile([128, 128], BF16, tag="ident")
        mask4 = consts.tile([128, 512], BF16, tag="mask4")

        qsb = [consts.tile([128, NCH * N], BF16, tag=f"q{h}", name=f"q{h}")
               for h in range(HPC)]
        vsb = [consts.tile([128, NCH * DV], BF16, tag=f"v{h}", name=f"v{h}")
               for h in range(HPC)]
        ctab = consts.tile([128, NCH * NP], BF16, tag="ctab")
        stab = consts.tile([128, NCH * NP], BF16, tag="stab")
        osb = consts.tile([128, HPC * NCH * DV], BF16, tag="osb")

        def load_q(h, c0, cl):
            lo, hi = (h * NCH + c0) * N, (h * NCH + c0 + cl) * N
            nc.sync.dma_start(qsb[h][:, c0 * N:(c0 + cl) * N], q_d[:, lo:hi])

        def load_v(h, c0, cl):
            lo, hi = (h * NCH + c0) * DV, (h * NCH + c0 + cl) * DV
            nc.sync.dma_start(vsb[h][:, c0 * DV:(c0 + cl) * DV], v_d[:, lo:hi])

        def load_tab(c0, cl):
            nc.sync.dma_start(ctab[:, c0 * NP:(c0 + cl) * NP],
                              c_d[:, c0 * NP:(c0 + cl) * NP])
            nc.sync.dma_start(stab[:, c0 * NP:(c0 + cl) * NP],
                              s_d[:, c0 * NP:(c0 + cl) * NP])

        # need-ordered loads (SP serial): early pieces small, later big.
        load_tab(0, 4)
        load_q(0, 0, 2)
        load_q(1, 0, 2)
        nc.sync.dma_start(ident[:, :], ident_d[:, :])
        load_v(0, 0, 4)
        load_v(1, 0, 4)
        load_q(0, 2, 2)
        load_q(1, 2, 2)
        nc.sync.dma_start(mask4[:, :], mask4_d[:, :])
        load_tab(4, 4)
        load_q(0, 4, 4)
        load_q(1, 4, 4)
        load_v(0, 4, 4)
        load_v(1, 4, 4)
        load_tab(8, 8)
        load_q(0, 8, 8)
        load_q(1, 8, 8)
        load_v(0, 8, 8)
        load_v(1, 8, 8)
        for h in (2, 3):
            load_q(h, 0, 8)
            load_v(h, 0, 16)
        for h in (2, 3):
            load_q(h, 8, 8)

        ctv = ctab[:, :].rearrange("p (c k) -> p c k", c=NCH)
        stv = stab[:, :].rearrange("p (c k) -> p c k", c=NCH)

        # rope engine schedule: 6 op slots per group
        #   [m1=qE*c, m2=qO*s, m3=qO*c, m4=qE*s, qrE=m1-m2, qrO=m3+m4]
        # DVE is cheapest (2x mode); Pool takes ~1.25 slots on average.
        rope_ctr = [0]

        def emit_rope(h, c0, cl, qr_tile):
            g = rope_ctr[0]
            rope_ctr[0] += 1
            qv = qsb[h][:, :].rearrange("p (c n) -> p c n", c=NCH)
            qE = qv[:, c0:c0 + cl, 0:NP]
            qO = qv[:, c0:c0 + cl, NP:N]
            cv = ctv[:, c0:c0 + cl, :]
            sv = stv[:, c0:c0 + cl, :]
            qrv = qr_tile[:, :].rearrange("p (c e k) -> p c e k", c=cl, e=2)
            qrE = qrv[:, :, 0, :]
            qrO = qrv[:, :, 1, :]

            def mt(tag):
                t = ropep.tile([128, cl * NP], BF16, tag=tag)
                return t[:, :].rearrange("p (c k) -> p c k", c=cl)

            m1, m2, m3, m4 = mt("m1"), mt("m2"), mt("m3"), mt("m4")
            # GPSIMD cannot touch PSUM, so Pool only ever does rope.  Keep
            # Pool's slots dependency-free (independent muls) so its chain
            # latency per group stays short; the dependent tail ops (sub/add)
            # run on DVE, with add alternating to Pool every other group to
            # balance totals.
            nc.gpsimd.tensor_mul(m1, qE, cv)
            nc.vector.tensor_mul(m2, qO, sv)
            nc.gpsimd.tensor_mul(m3, qO, cv)
            nc.gpsimd.tensor_mul(m4, qE, sv)
            nc.vector.tensor_sub(qrE, m1, m2)
            if g % 2 == 0:
                nc.gpsimd.tensor_add(qrO, m3, m4)
            else:
                nc.vector.tensor_add(qrO, m3, m4)

        # Global rope emission plan.  Slots: pass0 prologue-top=0,
        # prologue-bottom=1, iteration j bottom=2+j; pass1 shifted by 10.
        # Pass-1 groups are emitted EARLY (during pass-0 iterations, after
        # their q DMA lands) so Pool/DVE never sit on rope work at the end
        # and the pipeline drain stays short.
        rope_plan = {
            0: [(0, 0, 2), (0, 2, 2)],
            1: [(0, 4, 4)],
            3: [(0, 8, 8)],
            6: [(1, 0, 8)],
            8: [(1, 8, 8)],
        }
        qr_tiles = {}          # (pass_i, k, c) -> (tile, c0)
        qr_seq = [0]

        def emit_rope_slot(s):
            for (pi, c0, cl) in rope_plan.get(s, []):
                for k in range(2):
                    h = 2 * pi + k
                    t = qrp.tile([128, cl * N], BF16, tag=f"qr{k}",
                                 name=f"qr_{pi}_{k}_{c0}_{qr_seq[0]}")
                    qr_seq[0] += 1
                    emit_rope(h, c0, cl, t)
                    for c in range(c0, c0 + cl):
                        qr_tiles[(pi, k, c)] = (t, c0)

        # per pass: heads (2p, 2p+1) chunk-locked
        for pass_i in (0, 1):
            heads = (2 * pass_i, 2 * pass_i + 1)
            slot_base = pass_i * 10

            def qr_slice(k, c, half):
                t, c0 = qr_tiles[(pass_i, k, c)]
                v = t[:, :].rearrange("p (c e k) -> p c e k", c=(t.shape[1] // N), e=2)
                return v[:, c - c0, half, :]

            # transposes of block j (chunks 2j, 2j+1, both heads) go into one
            # 2KB bf16 psum bank; the evac is emitted separately so p-evacs
            # are never queued behind it on Act.
            qrt_sb = {}        # block -> sbuf tile [128, 1024]
            qrt_ps_t = {}      # block -> psum tile

            def emit_transposes(j):
                ps = ps_qrt.tile([128, 1024], BF16, tag="qrt_ps",
                                 name=f"qrtps_{pass_i}_{j}")
                for ci, c in enumerate((2 * j, 2 * j + 1)):
                    for k in range(2):
                        for half in range(2):
                            off = ((ci * 2 + k) * 2 + half) * 128
                            nc.tensor.matmul(
                                ps[:, off:off + 128], lhsT=qr_slice(k, c, half),
                                rhs=ident[:, :], is_transpose=True,
                                start=True, stop=True)
                qrt_ps_t[j] = ps

            def emit_qrt_evac(j):
                sb = qrtp.tile([128, 1024], BF16, tag="qrt_sb",
                               name=f"qrtsb_{pass_i}_{j}")
                if j % 2 == 0:
                    nc.scalar.copy(sb[:, :], qrt_ps_t[j][:, :])
                else:
                    nc.vector.tensor_copy(sb[:, :], qrt_ps_t[j][:, :])
                qrt_sb[j] = sb

            def qrt_slice(k, c, half):
                j = c // 2
                ci = c % 2
                off = ((ci * 2 + k) * 2 + half) * 128
                return qrt_sb[j][:, off:off + 128]

            p_ps = ps_p.tile([128, 256], F32, tag="pps", name=f"pps{pass_i}")
            o8 = [None, None]
            p_sb = {}          # chunk -> sbuf tile holding P after that chunk
            st_sb = {}         # block -> masked bf16 scores [128, 512]
            st_ps_t = {}       # block -> raw f32 scores in psum

            def emit_ST(j):
                st_ps = ps_st.tile([128, 512], F32, tag="st_ps",
                                   name=f"stps_{pass_i}_{j}")
                for ci, c in enumerate((2 * j, 2 * j + 1)):
                    for k in range(2):
                        sl = st_ps[:, (ci * 2 + k) * 128:(ci * 2 + k + 1) * 128]
                        nc.tensor.matmul(sl, lhsT=qrt_slice(k, c, 0),
                                         rhs=qrt_slice(k, c, 0),
                                         start=True, stop=False)
                        nc.tensor.matmul(sl, lhsT=qrt_slice(k, c, 1),
                                         rhs=qrt_slice(k, c, 1),
                                         start=False, stop=True)
                st_ps_t[j] = st_ps

            def emit_mask(j):
                sb = stp.tile([128, 512], BF16, tag="st_sb",
                              name=f"stsb_{pass_i}_{j}")
                nc.vector.tensor_mul(sb[:, :], st_ps_t[j][:, :], mask4[:, :])
                st_sb[j] = sb

            # P += QR_c^T V_c, both heads (shared bank, long-open group; only
            # the very first matmul of the pass starts it), then the pair P
            # evacuation on Act.  The p-evac -> next P-update WAR round trip
            # is the critical ring; callers place a full block of independent
            # PE work between consecutive emit_P calls.
            def emit_P(c):
                first = c == 0
                last = c == NCH - 1
                for k, h in enumerate(heads):
                    vi = vsb[h][:, c * DV:(c + 1) * DV]
                    for half in range(2):
                        reg = p_ps[:, k * 128 + half * 64:k * 128 + (half + 1) * 64]
                        nc.tensor.matmul(
                            reg, lhsT=qr_slice(k, c, half), rhs=vi,
                            start=(first and k == 0 and half == 0),
                            stop=last, skip_group_check=True)
                if not last:
                    p_new = pp.tile([128, 256], BF16, tag="p_sb",
                                    name=f"psb_{pass_i}_{c}")
                    nc.scalar.copy(p_new[:, :], p_ps[:, :])
                    p_sb[c] = p_new

            # out accumulation for block j (intra + inter) and the per-head
            # out evacuation + store.
            def emit_stage2(j):
                for ci, c in enumerate((2 * j, 2 * j + 1)):
                    first = c == 0
                    for k, h in enumerate(heads):
                        vi = vsb[h][:, c * DV:(c + 1) * DV]
                        if c % 8 == 0:
                            o8[k] = ps_o.tile([128, 512], F32, tag=f"o8_{k}",
                                              name=f"o8_{pass_i}_{k}_{c}")
                        o_sl = o8[k][:, (c % 8) * DV:(c % 8 + 1) * DV]
                        stm = st_sb[j][:, (ci * 2 + k) * 128:(ci * 2 + k + 1) * 128]
                        nc.tensor.matmul(o_sl, lhsT=stm, rhs=vi,
                                         start=True, stop=first)
                        if not first:
                            for half in range(2):
                                pv = p_sb[c - 1][:, k * 128 + half * 64:
                                                 k * 128 + (half + 1) * 64]
                                nc.tensor.matmul(
                                    o_sl, lhsT=qrt_slice(k, c, half), rhs=pv,
                                    start=False, stop=(half == 1),
                                    skip_group_check=True)
                        if c % 8 == 7:
                            g8 = c // 8
                            base = (h * NCH + g8 * 8) * DV
                            nc.scalar.copy(osb[:, base:base + 512], o8[k][:, :])
                        if c == NCH - 1:
                            base = h * NCH * DV
                            nc.sync.dma_start(o_d[:, base:base + 1024],
                                              osb[:, base:base + 1024])

            # software pipeline; PE stream per iteration j:
            #   T(j+2), ST(j+1), P(2j+2), intra/inter(j), P(2j+3)
            # so each P-update ring round-trip hides under independent work.
            emit_rope_slot(slot_base + 0)
            emit_transposes(0)
            emit_transposes(1)
            emit_qrt_evac(0)
            emit_qrt_evac(1)
            emit_ST(0)
            emit_mask(0)
            emit_P(0)
            emit_P(1)
            emit_rope_slot(slot_base + 1)
            NB = NCH // 2
            for j in range(NB):               # 8 blocks of 2 chunks
                if j + 2 < NB:
                    emit_transposes(j + 2)
                if j + 1 < NB:
                    emit_ST(j + 1)
                    emit_mask(j + 1)
                if j + 2 < NB:
                    emit_qrt_evac(j + 2)
                if j + 1 < NB:
                    emit_P(2 * j + 2)
                emit_stage2(j)
                if j + 1 < NB:
                    emit_P(2 * j + 3)
                emit_rope_slot(slot_base + 2 + j)

    nc.finalize()
    return nc


_NC = None


def _get_nc():
    global _NC
    if _NC is None:
        _NC = _build_nc()
    return _NC


def _host_prep(Q, V, freqs):
    """Host-side retiling to direct SBUF images.

    - Q feature axis permuted to [even | odd] halves (global relabeling of the
      contraction axis; scores/P invariant).
    - cos/sin pair tables [T, 128] (freqs are pair-quantized in the reference:
      floor(i/2)*2, so cos/sin agree within each (2i, 2i+1) pair).
    - every tensor stored as [128, free] so each DMA row is one contiguous
      descriptor run.
    """
    Qf = np.asarray(Q, dtype=np.float32).reshape(BH, T, N)
    Vf = np.asarray(V, dtype=np.float32).reshape(BH, T, DV)
    f = np.asarray(freqs, dtype=np.float32).reshape(N)

    t = np.arange(T, dtype=np.float32).reshape(T, 1)
    ang = np.mod(t * f.reshape(1, N), 1.0).astype(np.float32) * np.float32(2.0 * math.pi)
    ce = np.cos(ang[:, 0::2]).astype(NPBF16)     # [T, 128]
    se = np.sin(ang[:, 0::2]).astype(NPBF16)
    ctab = ce.reshape(NCH, CH, NP).transpose(1, 0, 2).reshape(128, NCH * NP)
    stab = se.reshape(NCH, CH, NP).transpose(1, 0, 2).reshape(128, NCH * NP)

    perm = np.concatenate([np.arange(0, N, 2), np.arange(1, N, 2)])
    Qp = Qf[:, :, perm].astype(NPBF16)           # [BH, T, N] -> E|O halves
    Vb = Vf.astype(NPBF16)

    q_cores = []
    v_cores = []
    for c in range(NCORES):
        hs = slice(c * HPC, (c + 1) * HPC)
        qc = Qp[hs].reshape(HPC, NCH, CH, N).transpose(2, 0, 1, 3).reshape(
            128, HPC * NCH * N)
        vc = Vb[hs].reshape(HPC, NCH, CH, DV).transpose(2, 0, 1, 3).reshape(
            128, HPC * NCH * DV)
        q_cores.append(np.ascontiguousarray(qc))
        v_cores.append(np.ascontiguousarray(vc))
    return q_cores, v_cores, np.ascontiguousarray(ctab), np.ascontiguousarray(stab)


def _run(inputs, trace=False, trace_kwargs=None):
    q_cores, v_cores, ctab, stab = _host_prep(
        inputs["Q"], inputs["V"], inputs["freqs"])

    in_maps = []
    for c in range(NCORES):
        in_maps.append({
            "q": q_cores[c],
            "v": v_cores[c],
            "ctab": ctab,
            "stab": stab,
        })

    nc = _get_nc()
    kw = {}
    if trace:
        kw = dict(trace=True, trace_kwargs=trace_kwargs or {})
    res = run_bass_kernel_spmd(nc, in_maps, core_ids=list(range(NCORES)), **kw)

    out = np.empty((BH, T, DV), dtype=np.float32)
    for c in range(NCORES):
        oc = res.results[c]["out"].astype(np.float32)        # [128, HPC*NCH*DV]
        oc = oc.reshape(128, HPC, NCH, DV).transpose(1, 2, 0, 3)
        out[c * HPC:(c + 1) * HPC] = oc.reshape(HPC, T, DV)
    return out.reshape(B, NH, T, DV), res


def kernel(**inputs):
    out, _ = _run(inputs, trace=False)
    return out
